# revision 1
# baseline (speedup 1.0000x reference)
"""Trainium2 Bass kernel for batched bilinear attention (sparse_attention).

Reference semantics (per batch b):
    hs_proj = hs @ W_a                      # [S, K]
    score[t,s] = ht[t,:] . hs_proj[s,:]     # = (ht @ W_a^T ... contraction over K)
    score -= rowmax(score)
    lens_b = count(source[b] != 0)
    e = exp(score) * (arange(S) < lens_b)
    a = e / rowsum(e)
    c = a @ hs
    out = tanh(concat([c, ht], -1) @ W_c + b)

Distribution: data-parallel over batch. B=16 across 8 cores -> 2 batches/core.
No collectives needed.

Per-core compute layout (per batch, T=S=H=O=1024, tiles of 128, chunks of 512):
    P[h, t]     = sum_k W_a[h,k] htT[k,t]        lhsT = W_aT tile, rhs = htT
    score[t, s] = sum_h P[h,t] hsT[h,s] + 1[t] * logmask[s]   (K=1 matmul adds mask)
    softmax over free dim s (rowmax via DVE, exp+rowsum via ACT accum, scale by 1/Z)
    aT[s, t]    = PE-transpose of a[t, s]
    cT[h, t]    = sum_s hs[s,h] aT[s,t]
    out[t, o]   = tanh( sum_h cT[h,t] Wc_top[h,o] + sum_h htT[h,t] Wc_bot[h,o]
                        + 1[t] * bias[o] )

All matmul operands are fp16 (full PE rate, 8x finer rounding than bf16;
measured end-to-end rel err 1.9e-3 vs 1.5e-2 for bf16); accumulation is always
fp32 in PSUM.
"""

from contextlib import ExitStack

import ml_dtypes
import numpy as np

import concourse.bass as bass
import concourse.tile as tile
from concourse import bacc, mybir
from concourse.bass_utils import run_bass_kernel_spmd
from concourse.masks import make_identity

# ---- problem constants (hardcoded per contract) ----
B, T, S, H, O = 16, 1024, 1024, 1024, 1024
NCORES = 8
BPC = B // NCORES  # batches per core
P = 128            # partition tile
NT = T // P        # 8 tiles per 1024 dim
CHUNK = 512        # free-dim chunk (one PSUM bank of fp32)
NCH = T // CHUNK   # 2 t-chunks per batch
NEG_BIG = -30000.0

F32 = mybir.dt.float32
# All matmul inputs are fp16: same PE rate and same 2-byte footprint as bf16,
# but 11 significand bits instead of 8 -- measured end-to-end rel err 1.9e-3 vs
# 1.5e-2 for bf16.  All values here are well inside fp16 range (inputs ~N(0,1),
# weights ~N(0,1/sqrt(dim)), attention weights in [0,1]); the additive log-mask
# uses -30000 (fp16-finite), which still underflows exp() to exactly 0.
MMDT = mybir.dt.float16
MMDT_NP = np.float16
# graph variants decided from the actual input data in kernel():
USE_BIAS = True          # emit the +bias K=1 matmuls (skipped when b is all-zero)
MASK_CHUNK = [True, True]  # emit the log-mask matmul for each 512-wide s-chunk
_NC_CACHE = {}
LAST_RESULT = None


def _build_kernel(ctx: ExitStack, tc: tile.TileContext, d):
    nc = tc.nc

    # ---------------- pools ----------------
    # weights: NT row-tiles per matrix, resident for the whole kernel
    w_pool = ctx.enter_context(tc.tile_pool(name="weights", bufs=NT))
    const_pool = ctx.enter_context(tc.tile_pool(name="consts", bufs=1))
    # per-batch inputs: NT row-tiles live + prefetch slots for the next batch
    htTin_pool = ctx.enter_context(tc.tile_pool(name="htTin", bufs=3 * NT))
    hsTin_pool = ctx.enter_context(tc.tile_pool(name="hsTin", bufs=4 * NT))
    hsin_pool = ctx.enter_context(tc.tile_pool(name="hsin", bufs=2 * NT))
    # per-chunk intermediates, double-buffered across chunks
    p_pool = ctx.enter_context(tc.tile_pool(name="psb", bufs=2 * NT))
    aT_pool = ctx.enter_context(tc.tile_pool(name="aT", bufs=2))
    cT_pool = ctx.enter_context(tc.tile_pool(name="cT", bufs=2 * NT))
    e_pool = ctx.enter_context(tc.tile_pool(name="e", bufs=3))
    stat_pool = ctx.enter_context(tc.tile_pool(name="stats", bufs=2))
    out_pool = ctx.enter_context(tc.tile_pool(name="outsb", bufs=3))

    # PSUM: 8 banks total -> pps 2 + sps 3 + tp 1 + mm2 2 = 8
    pps_pool = ctx.enter_context(tc.tile_pool(name="pps", bufs=2, space="PSUM"))
    sps_pool = ctx.enter_context(tc.tile_pool(name="sps", bufs=3, space="PSUM"))
    tp_pool = ctx.enter_context(tc.tile_pool(name="tp", bufs=1, space="PSUM"))
    mm2_pool = ctx.enter_context(tc.tile_pool(name="mm2", bufs=2, space="PSUM"))

    # ---------------- persistent weights / constants ----------------
    # DMA emission order is the DMA issue order, so stage it by first use:
    # (waT[i], htT_b0[i]) pairs feed the first P-matmuls within ~2 us, hsT_b0
    # feeds the first scores, hs_b0 feeds ctilde, and W_c (first needed ~80 us
    # in) goes last.
    def _load_htT_chunk(b, ch):
        tiles = []
        for i in range(NT):
            rsl = slice(i * P, (i + 1) * P)
            t = htTin_pool.tile([P, CHUNK], MMDT, tag="htT")   # [k_in, t-chunk]
            nc.sync.dma_start(t[:], d["htT"].ap()[b, ch, rsl, :])
            tiles.append(t)
        return tiles

    def _load_hsT(b):
        # [sc][hh] column-chunk tiles, sc-major so the first s-chunk's scores
        # only wait for 1 MB
        tiles = []
        for sc in range(S // CHUNK):
            row = []
            for hh in range(NT):
                t = hsTin_pool.tile([P, CHUNK], MMDT, tag="hsT")  # [h_in, s-chunk]
                nc.sync.dma_start(t[:], d["hsT"].ap()[b, sc, hh * P : (hh + 1) * P, :])
                row.append(t)
            tiles.append(row)
        return tiles

    def _load_batch_inputs(b, what):
        tiles = []
        for i in range(NT):
            rsl = slice(i * P, (i + 1) * P)
            t = hsin_pool.tile([P, H], MMDT, tag="hs")     # [s_in, h]
            nc.sync.dma_start(t[:], d[what].ap()[b, rsl, :])
            tiles.append(t)
        return tiles

    waT_t, htT_b0c0 = [], []
    for i in range(NT):
        rsl = slice(i * P, (i + 1) * P)
        t = w_pool.tile([P, H], MMDT, tag="waT")
        nc.sync.dma_start(t[:], d["waT"].ap()[rsl, :])
        waT_t.append(t)
        t = htTin_pool.tile([P, CHUNK], MMDT, tag="htT")
        nc.sync.dma_start(t[:], d["htT"].ap()[0, 0, rsl, :])
        htT_b0c0.append(t)
    hsT_b0 = _load_hsT(0)
    htT_b0 = [htT_b0c0, _load_htT_chunk(0, 1)]

    lm_sb = const_pool.tile([1, BPC, S], MMDT, tag="lm")
    nc.sync.dma_start(lm_sb[:], d["lm"].ap().rearrange("(x b) s -> x b s", x=1))
    ones_sb = const_pool.tile([1, P], MMDT, tag="ones")
    nc.vector.memset(ones_sb[:], 1.0)
    ident_sb = const_pool.tile([P, P], MMDT, tag="ident")
    make_identity(nc, ident_sb[:])

    hs_b0 = _load_batch_inputs(0, "hs")

    wcTop_t, wcBot_t = [], []
    for i in range(NT):
        rsl = slice(i * P, (i + 1) * P)
        t = w_pool.tile([P, O], MMDT, tag="wcTop")
        nc.sync.dma_start(t[:], d["wcTop"].ap()[rsl, :])
        wcTop_t.append(t)
        t = w_pool.tile([P, O], MMDT, tag="wcBot")
        nc.sync.dma_start(t[:], d["wcBot"].ap()[rsl, :])
        wcBot_t.append(t)
    bias_sb = const_pool.tile([1, O], MMDT, tag="bias")
    nc.sync.dma_start(bias_sb[:], d["bias"].ap())

    # all remaining batches' inputs are emitted upfront: every slot is free
    # (bufs=2*NT), so the sync-engine triggers fire early and the data lands
    # long before the batch boundary
    batch_inputs = {0: (htT_b0, hsT_b0, hs_b0)}
    for b in range(1, BPC):
        batch_inputs[b] = (
            [_load_htT_chunk(b, 0), _load_htT_chunk(b, 1)],
            _load_hsT(b),
            _load_batch_inputs(b, "hs"),
        )

    # ---------------- per-batch program ----------------
    # Flat (batch, chunk) iteration, software-pipelined: the NEXT chunk's
    # P-projection matmuls are emitted right after the LAST t-tile's score
    # matmuls, so the PE has work during that softmax's latency (otherwise the
    # in-order PE stream stalls ~3.6 us at every chunk boundary waiting for
    # the transposes' input).
    iters = [(b, ch) for b in range(BPC) for ch in range(NCH)]

    def compute_P(b, ch):
        htT_c = batch_inputs[b][0][ch]
        p_t = []
        for hh in range(NT):
            pps = pps_pool.tile([P, CHUNK], F32, tag="pps")
            for kt in range(NT):
                nc.tensor.matmul(
                    pps[:],
                    waT_t[kt][:, hh * P : (hh + 1) * P],
                    htT_c[kt][:],
                    start=(kt == 0),
                    stop=(kt == NT - 1),
                )
            pt = p_pool.tile([P, CHUNK], MMDT, tag="psb")
            nc.vector.tensor_copy(pt[:], pps[:])
            p_t.append(pt)
        return p_t

    p_t = compute_P(0, 0)
    for it, (b, ch) in enumerate(iters):
        htT_t, hsT_t, hs_t = batch_inputs[b]
        tlo = ch * CHUNK  # global t offset of this chunk

        # ---- per t-tile: score + softmax + transpose ----
        aT_sb = aT_pool.tile([P, NT, CHUNK], MMDT, tag="aT")  # [s_in, st, t]
        for tl in range(CHUNK // P):  # 4 t-tiles of 128 in the 512 chunk
            tsl = slice(tl * P, (tl + 1) * P)

            sps_list = []
            for sc in range(S // CHUNK):
                ssl = slice(sc * CHUNK, (sc + 1) * CHUNK)
                sps = sps_pool.tile([P, CHUNK], F32, tag="sps")
                for hh in range(NT):
                    nc.tensor.matmul(
                        sps[:],
                        p_t[hh][:, tsl],
                        hsT_t[sc][hh][:],
                        start=(hh == 0),
                        stop=(hh == NT - 1) and not MASK_CHUNK[sc],
                    )
                if MASK_CHUNK[sc]:
                    # add log-mask row: score += ones[t] * lm[s]
                    nc.tensor.matmul(
                        sps[:],
                        ones_sb[:, :],
                        lm_sb[:, b, ssl],
                        start=False,
                        stop=True,
                    )
                sps_list.append(sps)

            # softmax over s (free dim), chunked.  One stat tile per t-tile:
            # cols 0:m0 1:m1 2:negm 3:z0 4:z1 5:rz
            st_t = stat_pool.tile([P, 6], F32, tag="stat")
            nc.vector.tensor_reduce(st_t[:, 0:1], sps_list[0][:], axis=mybir.AxisListType.X, op=mybir.AluOpType.max, negate=True)
            nc.vector.tensor_reduce(st_t[:, 1:2], sps_list[1][:], axis=mybir.AxisListType.X, op=mybir.AluOpType.max, negate=True)
            # min of negated maxes = -(overall max): feeds exp bias directly
            nc.vector.tensor_tensor(st_t[:, 2:3], st_t[:, 0:1], st_t[:, 1:2], op=mybir.AluOpType.min)

            e_sb = e_pool.tile([P, S], MMDT, tag="e")
            nc.scalar.activation(
                e_sb[:, 0:CHUNK], sps_list[0][:], mybir.ActivationFunctionType.Exp,
                bias=st_t[:, 2:3], scale=1.0, accum_out=st_t[:, 3:4],
            )
            nc.scalar.activation(
                e_sb[:, CHUNK:S], sps_list[1][:], mybir.ActivationFunctionType.Exp,
                bias=st_t[:, 2:3], scale=1.0, accum_out=st_t[:, 4:5],
            )
            nc.vector.tensor_tensor(st_t[:, 5:6], st_t[:, 3:4], st_t[:, 4:5], op=mybir.AluOpType.add)
            nc.vector.reciprocal(st_t[:, 5:6], st_t[:, 5:6])

            a_sb = e_sb  # scaled in place: a = e * (1/Z)
            nc.vector.tensor_scalar_mul(a_sb[:], e_sb[:], st_t[:, 5:6])

            if tl == CHUNK // P - 1 and it + 1 < len(iters):
                # fill this (un-hidable) softmax latency with next chunk's P
                p_next = compute_P(*iters[it + 1])

            # aT[s, t-tile] via PE transpose; 4 transposes per PSUM bank,
            # then one wide strided copy out
            for g in range(2):
                tp = tp_pool.tile([P, 4, P], MMDT, tag="tp")
                for j in range(4):
                    st = g * 4 + j
                    nc.tensor.transpose(tp[:, j], a_sb[:, st * P : (st + 1) * P], ident_sb[:])
                nc.vector.tensor_copy(aT_sb[:, g * 4 : (g + 1) * 4, tsl], tp[:])

        # ---- cT[h, t-chunk] = hs @ aT ----
        cT_t = []
        for hh in range(NT):
            cps = mm2_pool.tile([P, CHUNK], F32, tag="mm2")
            for st in range(NT):
                nc.tensor.matmul(
                    cps[:],
                    hs_t[st][:, hh * P : (hh + 1) * P],
                    aT_sb[:, st, :],
                    start=(st == 0),
                    stop=(st == NT - 1),
                )
            ct = cT_pool.tile([P, CHUNK], MMDT, tag="cT")
            if it == len(iters) - 1:
                nc.vector.tensor_copy(ct[:, 0:CHUNK // 2], cps[:, 0:CHUNK // 2])
                nc.scalar.copy(ct[:, CHUNK // 2 :], cps[:, CHUNK // 2 :])
            elif hh % 2 == 0:
                nc.vector.tensor_copy(ct[:], cps[:])
            else:
                nc.scalar.copy(ct[:], cps[:])
            cT_t.append(ct)

        # ---- out[t, o] = tanh(cT.T @ WcTop + htT.T @ WcBot [+ bias]) ----
        for tl in range(CHUNK // P):
            tsl = slice(tl * P, (tl + 1) * P)
            gsl = slice(tlo + tl * P, tlo + (tl + 1) * P)
            for oc in range(O // CHUNK):
                osl = slice(oc * CHUNK, (oc + 1) * CHUNK)
                ops = mm2_pool.tile([P, CHUNK], F32, tag="mm2")
                for hh in range(NT):
                    nc.tensor.matmul(
                        ops[:],
                        cT_t[hh][:, tsl],
                        wcTop_t[hh][:, osl],
                        start=(hh == 0),
                        stop=False,
                    )
                for hh in range(NT):
                    nc.tensor.matmul(
                        ops[:],
                        htT_t[ch][hh][:, tsl],
                        wcBot_t[hh][:, osl],
                        start=False,
                        stop=(hh == NT - 1) and not USE_BIAS,
                    )
                if USE_BIAS:
                    nc.tensor.matmul(
                        ops[:], ones_sb[:, :], bias_sb[:, osl],
                        start=False, stop=True,
                    )
                out_sb = out_pool.tile([P, CHUNK], F32, tag="out")
                nc.scalar.activation(out_sb[:], ops[:], mybir.ActivationFunctionType.Tanh)
                nc.sync.dma_start(d["out"].ap()[b, gsl, osl], out_sb[:])

        if it + 1 < len(iters):
            p_t = p_next


def _get_nc():
    key = (USE_BIAS, tuple(MASK_CHUNK))
    if key in _NC_CACHE:
        return _NC_CACHE[key]

    nc = bacc.Bacc("TRN2", target_bir_lowering=False, debug=False)
    d = {
        "htT": nc.dram_tensor("htT", [BPC, NCH, H, CHUNK], MMDT, kind="ExternalInput"),
        "hsT": nc.dram_tensor("hsT", [BPC, S // CHUNK, H, CHUNK], MMDT, kind="ExternalInput"),
        "hs": nc.dram_tensor("hs", [BPC, S, H], MMDT, kind="ExternalInput"),
        "waT": nc.dram_tensor("waT", [H, H], MMDT, kind="ExternalInput"),
        "wcTop": nc.dram_tensor("wcTop", [H, O], MMDT, kind="ExternalInput"),
        "wcBot": nc.dram_tensor("wcBot", [H, O], MMDT, kind="ExternalInput"),
        "bias": nc.dram_tensor("bias", [1, O], MMDT, kind="ExternalInput"),
        "lm": nc.dram_tensor("lm", [BPC, S], MMDT, kind="ExternalInput"),
        "out": nc.dram_tensor("out", [BPC, T, O], F32, kind="ExternalOutput"),
    }
    with tile.TileContext(nc) as tc:
        with ExitStack() as ctx:
            _build_kernel(ctx, tc, d)
    nc.compile()
    _dedup_ldweights(nc)
    _NC_CACHE[key] = nc
    return nc


def _dedup_ldweights(nc):
    """Drop an InstLdweights when the PE's weight registers already hold the
    same operand (same AP, loaded by the immediately preceding LDWEIGHTS) and
    the instruction carries no semaphore waits/updates.  The paired matmuls
    then reuse the loaded weights, saving the un-hidden ~27ns FWL load."""
    ndrop = 0
    for f in nc.m.functions:
        for bb in f.blocks:
            insts = list(bb.instructions)
            new = []
            last_w = None
            for i in insts:
                if getattr(i, "engine", None) == mybir.EngineType.PE:
                    tn = type(i).__name__
                    if tn == "InstLdweights":
                        ap = i.ins[0]
                        k = (
                            str(getattr(ap, "memref", "")),
                            getattr(ap, "offset", None),
                            str(getattr(ap, "ap", "")),
                            str(getattr(ap, "dtype", "")),
                            str(getattr(i, "is_transpose", None)),
                        )
                        if k == last_w and not i.has_wait() and not i.has_update():
                            ndrop += 1
                            continue
                        last_w = k
                new.append(i)
            if len(new) != len(insts):
                bb.instructions = new
    return ndrop


def kernel(ht, hs, W_a, W_c, b, source):
    global LAST_RESULT
    ht = np.asarray(ht, dtype=np.float32)
    hs = np.asarray(hs, dtype=np.float32)
    W_a = np.asarray(W_a, dtype=np.float32)
    W_c = np.asarray(W_c, dtype=np.float32)
    b = np.asarray(b, dtype=np.float32)
    source = np.asarray(source)

    # host-side layout prep (sharding + per-layout copies); htT/hsT are stored
    # chunk-major so every SBUF tile load is one contiguous 128 KB DMA
    htT_f = ht.transpose(0, 2, 1)                                      # [B, H, T] fp32
    hsT_f = hs.transpose(0, 2, 1)                                      # [B, H, S] fp32
    htT = np.ascontiguousarray(
        htT_f.reshape(B, H, NCH, CHUNK).transpose(0, 2, 1, 3)
    ).astype(MMDT_NP)                                                  # [B, NCH, H, CHUNK]
    hsT = np.ascontiguousarray(
        hsT_f.reshape(B, H, S // CHUNK, CHUNK).transpose(0, 2, 1, 3)
    ).astype(MMDT_NP)                                                  # [B, S/CHUNK, H, CHUNK]
    hs_b = hs.astype(MMDT_NP)
    waT = np.ascontiguousarray(W_a.T).astype(MMDT_NP)                  # [K, H]
    wcTop = np.ascontiguousarray(W_c[:H]).astype(MMDT_NP)
    wcBot = np.ascontiguousarray(W_c[H:]).astype(MMDT_NP)
    bias = b.reshape(1, O).astype(MMDT_NP)

    lens = (source != 0).sum(axis=1)                                   # [B]
    lm = np.where(np.arange(S)[None, :] < lens[:, None], 0.0, NEG_BIG).astype(MMDT_NP)

    # graph-variant flags from the actual data (same graph on all cores)
    global USE_BIAS, MASK_CHUNK
    USE_BIAS = bool(np.any(b != 0))
    MASK_CHUNK = [bool(np.any(lens < (sc + 1) * CHUNK)) for sc in range(S // CHUNK)]

    in_maps = []
    for c in range(NCORES):
        sl = slice(c * BPC, (c + 1) * BPC)
        m = {
            "htT": htT[sl],
            "hsT": hsT[sl],
            "hs": hs_b[sl],
            "waT": waT,
            "wcTop": wcTop,
            "wcBot": wcBot,
            "bias": bias,
            "lm": lm[sl],
        }
        in_maps.append(m)

    nc = _get_nc()
    try:
        res = run_bass_kernel_spmd(nc, in_maps, core_ids=list(range(NCORES)))
    except Exception:
        # transient device errors (e.g. NRT_EXEC_UNIT_UNRECOVERABLE) occur
        # occasionally on the tunneled cores; one retry usually clears them
        import time as _time
        _time.sleep(5)
        res = run_bass_kernel_spmd(nc, in_maps, core_ids=list(range(NCORES)))
    LAST_RESULT = res
    out = np.concatenate([r["out"] for r in res.results], axis=0)
    return np.ascontiguousarray(out.astype(np.float32))

